# revision 9
# baseline (speedup 1.0000x reference)
"""Trainium2 Bass kernel for nn_CoreGroupConstruction (segment_reduce).

Reference loss: S = Wm @ exp(P) with Wm row-normalized masked seed weights
([8192, 2048]), P [2048, 2048] edge-independent; pointwise Bernoulli NLL over
all (edge, node) pairs + degree/size moment losses on row/col sums of S.

Algorithmic restructure (vs dense K=2048 matmul): P is bilinear in the K=32
binary attributes: P[i,j] = C + u_i + u_j + sum_k a_ik c_k a_jk, so
E = exp(P) = e^C diag(e^u) (1 + X + O(X^2)) diag(e^u) with X = A diag(c) A^T
of inner rank 32. First-order Taylor in X gives S = Wm @ E as a rank-33
product Z @ AT with Z = [s_e | (W~ A) diag(c)] * e^C and AT = [1 | A^T] row-
scaled by e^{u_j}. The Taylor truncation only perturbs terms ~8 orders of
magnitude below the 2e-2 tolerance (masked log S is dominated by the exact
host-side Wm in the blend constant; off-group S ~ 1e-10).

Device per core (1024 edges, M sharded 8 ways), per 128-edge tile:
  PSUM[128, 2048] = Z_et @ AT          (K=64 bf16 matmul, 4 bank-slices)
                  + I @ q_et           (fp8 DoubleRow copy-matmul accumulates
                                        the host blend constant into PSUM)
  DVE:  p = PSUM[:, :1024] * PSUM[:, 1024:]   (pair product)
  ACT:  Ln(p) with per-partition accumulate  (ln B1 + ln B2 = ln B1*B2)
q[e,j] = 2^s*(1 - Ic + Wm*(1 - E1_jj)) makes B = PSUM equal 2^s*S on-group
and 2^s*(1 + S') off-group (log(1+S') vs log(1-S') differs by ~4e-3 absolute
on a 4.1e6 loss).

Row/col sums of S (degree/size moments) are exact on host by associativity;
host gathers the 8 per-core loss partials and assembles the scalar.
"""

import numpy as np
import ml_dtypes

import concourse.bacc as bacc
import concourse.tile as tile
from concourse import mybir
from concourse.bass_utils import run_bass_kernel_spmd

M, NC, K = 8192, 2048, 32
N_CORES = 8
MLOC = M // N_CORES          # 1024 edges per core
P_DIM = 128
ET = MLOC // P_DIM           # 8 edge tiles per core
JBLK = 512                   # one f32 PSUM bank
NJ = NC // JBLK              # 4 j-slices
KZ = 64                      # padded contraction dim for the low-rank matmul
S_EXP = 6                    # global 2^6 scale keeps fp8/Ln operands in range
HNC = NC // 2                # pair-product width

_BF16 = ml_dtypes.bfloat16

_cache = {}


def _build_bass():
    nc = bacc.Bacc("TRN2", target_bir_lowering=False, debug=False)
    bf16 = mybir.dt.bfloat16
    fp8 = mybir.dt.float8e4
    f32 = mybir.dt.float32

    zz_d = nc.dram_tensor("zz", [KZ, ET * P_DIM], bf16, kind="ExternalInput")
    aa_d = nc.dram_tensor("aa", [KZ, NC], bf16, kind="ExternalInput")
    # DoubleRow-interleaved identity (k = r*64 + p) for the right-half blend
    ii_d = nc.dram_tensor("ii", [KZ, 2, P_DIM], fp8, kind="ExternalInput")
    # blend constants: left j-half for the DVE add, right j-half DR for the PE
    ql_d = nc.dram_tensor("ql", [ET, P_DIM, HNC], bf16, kind="ExternalInput")
    qr_d = nc.dram_tensor("qr", [ET, KZ, 2, HNC], fp8, kind="ExternalInput")
    loss_d = nc.dram_tensor("loss_pp", [P_DIM, ET], f32, kind="ExternalOutput")

    with tile.TileContext(nc) as tc:
        with (
            tc.tile_pool(name="const", bufs=1) as cpool,
            tc.tile_pool(name="qp", bufs=3) as qpool,
            tc.tile_pool(name="blp", bufs=3) as blpool,
            tc.tile_pool(name="scr", bufs=3) as spool,
            tc.tile_pool(name="psum", bufs=2, space="PSUM") as pspool,
        ):
            loss_pp = cpool.tile([P_DIM, ET], f32, tag="loss")
            zz_t = cpool.tile([KZ, ET * P_DIM], bf16, tag="zz")
            aa_t = cpool.tile([KZ, NC], bf16, tag="aa")
            ii_t = cpool.tile([KZ, 2, P_DIM], fp8, tag="ii")
            # critical-path first: zz + first aa slice gate the first matmul
            nc.sync.dma_start(zz_t[:], zz_d[:])
            nc.sync.dma_start(aa_t[:, 0:JBLK], aa_d[:, 0:JBLK])
            nc.sync.dma_start(ii_t[:], ii_d[:])
            for jb in range(1, NJ):
                nc.sync.dma_start(
                    aa_t[:, jb * JBLK:(jb + 1) * JBLK],
                    aa_d[:, jb * JBLK:(jb + 1) * JBLK],
                )

            for et in range(ET):
                qlt = qpool.tile([P_DIM, HNC], bf16, tag="ql")
                nc.gpsimd.dma_start(qlt[:], ql_d[et])
                qrt = qpool.tile([KZ, 2, HNC], fp8, tag="qr")
                nc.scalar.dma_start(qrt[:], qr_d[et])

                psb = pspool.tile([P_DIM, NC], f32, tag="ps")
                for jb in range(NJ):
                    nc.tensor.matmul(
                        psb[:, jb * JBLK:(jb + 1) * JBLK],
                        zz_t[:, et * P_DIM:(et + 1) * P_DIM],
                        aa_t[:, jb * JBLK:(jb + 1) * JBLK],
                        start=True,
                        stop=(jb < 2),
                    )
                for jb in range(2):
                    nc.tensor.matmul(
                        psb[:, HNC + jb * JBLK:HNC + (jb + 1) * JBLK],
                        ii_t[:],
                        qrt[:, :, jb * JBLK:(jb + 1) * JBLK],
                        start=False,
                        stop=True,
                        perf_mode=mybir.MatmulPerfMode.DoubleRow,
                    )
                # left half: DVE blend add; right half blended by the PE.
                # pair product: ln B_L + ln B_R = ln(B_L * B_R)
                blt = blpool.tile([P_DIM, HNC], bf16, tag="bl")
                nc.vector.tensor_add(blt[:], psb[:, 0:HNC], qlt[:])
                scr = spool.tile([P_DIM, HNC], bf16, tag="scr")
                nc.vector.tensor_mul(scr[:], psb[:, HNC:NC], blt[:])
                nc.scalar.activation(
                    scr[:], scr[:], mybir.ActivationFunctionType.Ln,
                    accum_out=loss_pp[:, et:et + 1],
                )

            nc.sync.dma_start(loss_d[:], loss_pp[:])
    nc.compile()
    return nc


def _host_precompute(theta_log, seed_prob, Ic, c2a):
    theta = -np.logaddexp(0.0, -theta_log.astype(np.float64))  # log_sigmoid [K,3]
    t0, t1, t2 = theta[:, 0], theta[:, 1], theta[:, 2]
    A = c2a.astype(np.float64)
    nA = 1.0 - A
    P = (nA * t0) @ nA.T + (A * t1) @ nA.T + (nA * t1) @ A.T + (A * t2) @ A.T
    np.fill_diagonal(P, 0.0)
    E = np.exp(P)                                # [NC, NC], diag == 1 (exact)

    sp = seed_prob.astype(np.float64)
    seed = np.exp(sp - sp.max())
    seed /= seed.sum()
    Icf = Ic.astype(np.float64)
    rs = Icf @ seed                              # [M]
    Wm = (Icf * seed[None, :]) / rs[:, None]     # [M, NC]

    # rank-33 factorization pieces (see module docstring)
    Cc = t0.sum()
    u = A @ (t1 - t0)                            # [NC]
    c = t0 + t2 - 2.0 * t1                       # [K]
    eu = np.exp(u)
    Wt = Wm * eu[None, :]
    scale = np.exp(Cc) * (2.0 ** S_EXP)
    Z = np.zeros((M, KZ), np.float64)
    Z[:, 0] = Wt.sum(axis=1) * scale
    Z[:, 1:K + 1] = (Wt @ A) * c[None, :] * scale
    AT = np.zeros((KZ, NC), np.float64)
    AT[0, :] = eu
    AT[1:K + 1, :] = A.T * eu[None, :]

    E1_jj = np.exp(Cc + 2.0 * u) * (1.0 + A @ c)     # Taylor-1 diag of E
    q = (2.0 ** S_EXP) * (1.0 - Icf + Wm * (1.0 - E1_jj)[None, :])
    return E, Wm, Icf, Z, AT, q


def _make_in_maps(Z, AT, q):
    fp8_np = mybir.dt.np(mybir.dt.float8e4)
    aa_np = np.ascontiguousarray(AT).astype(_BF16)
    # DoubleRow identity: ii[p, r, m] = 1 iff m == r*64 + p
    ii_np = np.zeros((KZ, 2, P_DIM), np.float32)
    for r in range(2):
        ii_np[np.arange(KZ), r, r * KZ + np.arange(KZ)] = 1.0
    ii_np = ii_np.astype(fp8_np)
    in_maps = []
    for cid in range(N_CORES):
        sl = slice(cid * MLOC, (cid + 1) * MLOC)
        zz_np = np.ascontiguousarray(Z[sl].T).astype(_BF16)           # [KZ, 1024]
        qc = q[sl]
        ql_np = np.ascontiguousarray(
            qc[:, :HNC].reshape(ET, P_DIM, HNC)
        ).astype(_BF16)
        # qr[et, p, r, j] = q[et*128 + r*64 + p, HNC + j]
        qr_np = np.ascontiguousarray(
            qc[:, HNC:].reshape(ET, 2, KZ, HNC).transpose(0, 2, 1, 3)
        ).astype(fp8_np)
        in_maps.append({"zz": zz_np, "aa": aa_np, "ii": ii_np,
                        "ql": ql_np, "qr": qr_np})
    return in_maps


def kernel(theta_log, seed_prob, Ic, c2a):
    assert Ic.shape == (M, NC) and c2a.shape == (NC, K)
    E, Wm, Icf, Z, AT, q = _host_precompute(theta_log, seed_prob, Ic, c2a)
    in_maps = _make_in_maps(Z, AT, q)

    if "nc" not in _cache:
        _cache["nc"] = _build_bass()
    res = run_bass_kernel_spmd(_cache["nc"], in_maps, core_ids=list(range(N_CORES)))

    # device computed sum ln(2^S_EXP * blend) over all (e, j)
    loss_raw = sum(r["loss_pp"].astype(np.float64).sum() for r in res.results)
    loss = -(loss_raw - M * NC * S_EXP * np.log(2.0))
    # row/col sums of S, exact by associativity (f64)
    deg = Wm.sum(axis=0) @ E                     # [NC]
    sizes = Wm @ E.sum(axis=1)                   # [M]
    degree_exp = np.sort(deg)[::-1]
    size_exp = np.sort(sizes)[::-1]
    degree_ans = np.sort(Icf.sum(axis=0))[::-1]
    size_ans = np.sort(Icf.sum(axis=1))[::-1]
    degree_loss = np.mean((degree_exp - degree_ans) ** 2)
    size_loss = np.mean((size_exp - size_ans) ** 2)
    return np.float32(loss + degree_loss + size_loss)


# revision 14
# speedup vs baseline: 1.2380x; 1.2380x over previous
"""Trainium2 Bass kernel for nn_CoreGroupConstruction (segment_reduce).

Reference loss: S = Wm @ exp(P) with Wm row-normalized masked seed weights
([8192, 2048]), P [2048, 2048] edge-independent; pointwise Bernoulli NLL over
all (edge, node) pairs + degree/size moment losses on row/col sums of S.

Algorithmic restructure (vs dense K=2048 matmul): P is bilinear in the K=32
binary attributes: P[i,j] = C + u_i + u_j + sum_k a_ik c_k a_jk, so
E = exp(P) = e^C diag(e^u) (1 + X + O(X^2)) diag(e^u) with X = A diag(c) A^T
of inner rank 32. First-order Taylor in X gives S = Wm @ E as a rank-33
product Z @ AT with Z = [s_e | (W~ A) diag(c)] * e^C and AT = [1 | A^T] row-
scaled by e^{u_j}. The Taylor truncation only perturbs terms ~8 orders of
magnitude below the 2e-2 tolerance (masked log S is dominated by the exact
host-side Wm in the blend constant; off-group S ~ 1e-10).

Device per core (1024 edges, M sharded 8 ways), per 128-edge tile:
  PSUM[128, 2048] = Z_et @ AT          (K=64 bf16 matmul, 4 bank-slices)
                  + I @ q_et           (fp8 DoubleRow copy-matmul accumulates
                                        the host blend constant into PSUM)
  DVE:  p = PSUM[:, :1024] * PSUM[:, 1024:]   (pair product)
  ACT:  Ln(p) with per-partition accumulate  (ln B1 + ln B2 = ln B1*B2)
q[e,j] = 2^s*(1 - Ic + Wm*(1 - E1_jj)) makes B = PSUM equal 2^s*S on-group
and 2^s*(1 + S') off-group (log(1+S') vs log(1-S') differs by ~4e-3 absolute
on a 4.1e6 loss).

Row/col sums of S (degree/size moments) are exact on host by associativity;
host gathers the 8 per-core loss partials and assembles the scalar.
"""

import numpy as np
import ml_dtypes

import concourse.bacc as bacc
import concourse.tile as tile
from concourse import mybir
from concourse.bass_utils import run_bass_kernel_spmd

M, NC, K = 8192, 2048, 32
N_CORES = 8
MLOC = M // N_CORES          # 1024 edges per core
P_DIM = 128
ET = MLOC // P_DIM           # 8 edge tiles per core
JBLK = 512                   # one f32 PSUM bank
NJ = NC // JBLK              # 4 j-slices
KZ = 64                      # padded contraction dim for the low-rank matmul
S_EXP = 6                    # global 2^6 scale keeps fp8/Ln operands in range
HNC = NC // 2                # pair-product width

_BF16 = ml_dtypes.bfloat16

_cache = {}


def _build_bass():
    nc = bacc.Bacc("TRN2", target_bir_lowering=False, debug=False)
    bf16 = mybir.dt.bfloat16
    fp8 = mybir.dt.float8e4
    f32 = mybir.dt.float32

    zz_d = nc.dram_tensor("zz", [KZ, ET * P_DIM], bf16, kind="ExternalInput")
    aa_d = nc.dram_tensor("aa", [KZ, NC], bf16, kind="ExternalInput")
    # DoubleRow-interleaved identity (k = r*64 + p) for the right-half blend
    ii_d = nc.dram_tensor("ii", [KZ, 2, P_DIM], fp8, kind="ExternalInput")
    # blend constants: left j-half for the DVE add, right j-half DR for the PE
    ql_d = nc.dram_tensor("ql", [ET, P_DIM, HNC], bf16, kind="ExternalInput")
    qr_d = nc.dram_tensor("qr", [ET, KZ, 2, 2, JBLK], fp8, kind="ExternalInput")
    loss_d = nc.dram_tensor("loss_pp", [P_DIM, ET], f32, kind="ExternalOutput")

    with tile.TileContext(nc) as tc:
        with (
            tc.tile_pool(name="const", bufs=1) as cpool,
            tc.tile_pool(name="qp", bufs=3) as qpool,
            tc.tile_pool(name="blp", bufs=3) as blpool,
            tc.tile_pool(name="scr", bufs=3) as spool,
            tc.tile_pool(name="psum", bufs=2, space="PSUM") as pspool,
        ):
            loss_pp = cpool.tile([P_DIM, ET], f32, tag="loss")
            zz_t = cpool.tile([KZ, ET * P_DIM], bf16, tag="zz")
            aa_t = cpool.tile([KZ, NC], bf16, tag="aa")
            ii_t = cpool.tile([KZ, 2, P_DIM], fp8, tag="ii")
            # critical-path first: zz + left aa half gate the first matmuls
            nc.sync.dma_start(zz_t[:], zz_d[:])
            nc.sync.dma_start(aa_t[:, 0:HNC], aa_d[:, 0:HNC])
            nc.sync.dma_start(ii_t[:], ii_d[:])
            nc.sync.dma_start(aa_t[:, HNC:NC], aa_d[:, HNC:NC])

            for et in range(ET):
                qlt = qpool.tile([P_DIM, HNC], bf16, tag="ql")
                nc.gpsimd.dma_start(qlt[:], ql_d[et])
                qrt = qpool.tile([KZ, 2, 2, JBLK], fp8, tag="qr")
                nc.scalar.dma_start(qrt[:], qr_d[et])

                psl = pspool.tile([P_DIM, HNC], f32, tag="psl")
                psr = pspool.tile([P_DIM, HNC], f32, tag="psr")
                for jb in range(2):
                    nc.tensor.matmul(
                        psl[:, jb * JBLK:(jb + 1) * JBLK],
                        zz_t[:, et * P_DIM:(et + 1) * P_DIM],
                        aa_t[:, jb * JBLK:(jb + 1) * JBLK],
                        start=True,
                        stop=True,
                    )
                for jb in range(2):
                    nc.tensor.matmul(
                        psr[:, jb * JBLK:(jb + 1) * JBLK],
                        zz_t[:, et * P_DIM:(et + 1) * P_DIM],
                        aa_t[:, HNC + jb * JBLK:HNC + (jb + 1) * JBLK],
                        start=True,
                        stop=False,
                    )
                for jb in range(2):
                    nc.tensor.matmul(
                        psr[:, jb * JBLK:(jb + 1) * JBLK],
                        ii_t[:],
                        qrt[:, jb],
                        start=False,
                        stop=True,
                        perf_mode=mybir.MatmulPerfMode.DoubleRow,
                    )
                # left half: DVE blend add; right half blended by the PE.
                # pair product: ln B_L + ln B_R = ln(B_L * B_R)
                blt = blpool.tile([P_DIM, HNC], bf16, tag="bl")
                nc.vector.tensor_add(blt[:], psl[:], qlt[:])
                scr = spool.tile([P_DIM, HNC], bf16, tag="scr")
                nc.vector.tensor_mul(scr[:], psr[:], blt[:])
                nc.scalar.activation(
                    scr[:], scr[:], mybir.ActivationFunctionType.Ln,
                    accum_out=loss_pp[:, et:et + 1],
                )

            nc.sync.dma_start(loss_d[:], loss_pp[:])
    nc.compile()
    return nc


def _host_precompute(theta_log, seed_prob, Ic, c2a):
    theta = -np.logaddexp(0.0, -theta_log.astype(np.float64))  # log_sigmoid [K,3]
    t0, t1, t2 = theta[:, 0], theta[:, 1], theta[:, 2]
    A = c2a.astype(np.float64)
    nA = 1.0 - A
    P = (nA * t0) @ nA.T + (A * t1) @ nA.T + (nA * t1) @ A.T + (A * t2) @ A.T
    np.fill_diagonal(P, 0.0)
    E = np.exp(P)                                # [NC, NC], diag == 1 (exact)

    sp = seed_prob.astype(np.float64)
    seed = np.exp(sp - sp.max())
    seed /= seed.sum()
    Icf = Ic.astype(np.float64)
    rs = Icf @ seed                              # [M]
    Wm = (Icf * seed[None, :]) / rs[:, None]     # [M, NC]

    # rank-33 factorization pieces (see module docstring)
    Cc = t0.sum()
    u = A @ (t1 - t0)                            # [NC]
    c = t0 + t2 - 2.0 * t1                       # [K]
    eu = np.exp(u)
    Wt = Wm * eu[None, :]
    scale = np.exp(Cc) * (2.0 ** S_EXP)
    Z = np.zeros((M, KZ), np.float64)
    Z[:, 0] = Wt.sum(axis=1) * scale
    Z[:, 1:K + 1] = (Wt @ A) * c[None, :] * scale
    AT = np.zeros((KZ, NC), np.float64)
    AT[0, :] = eu
    AT[1:K + 1, :] = A.T * eu[None, :]

    E1_jj = np.exp(Cc + 2.0 * u) * (1.0 + A @ c)     # Taylor-1 diag of E
    q = (2.0 ** S_EXP) * (1.0 - Icf + Wm * (1.0 - E1_jj)[None, :])
    return E, Wm, Icf, Z, AT, q


def _make_in_maps(Z, AT, q):
    fp8_np = mybir.dt.np(mybir.dt.float8e4)
    aa_np = np.ascontiguousarray(AT).astype(_BF16)
    # DoubleRow identity: ii[p, r, m] = 1 iff m == r*64 + p
    ii_np = np.zeros((KZ, 2, P_DIM), np.float32)
    for r in range(2):
        ii_np[np.arange(KZ), r, r * KZ + np.arange(KZ)] = 1.0
    ii_np = ii_np.astype(fp8_np)
    in_maps = []
    for cid in range(N_CORES):
        sl = slice(cid * MLOC, (cid + 1) * MLOC)
        zz_np = np.ascontiguousarray(Z[sl].T).astype(_BF16)           # [KZ, 1024]
        qc = q[sl]
        ql_np = np.ascontiguousarray(
            qc[:, :HNC].reshape(ET, P_DIM, HNC)
        ).astype(_BF16)
        # qr[et, p, jb, r, j] = q[et*128 + r*64 + p, HNC + jb*512 + j]
        qr_np = np.ascontiguousarray(
            qc[:, HNC:].reshape(ET, 2, KZ, 2, JBLK).transpose(0, 2, 3, 1, 4)
        ).astype(fp8_np)
        in_maps.append({"zz": zz_np, "aa": aa_np, "ii": ii_np,
                        "ql": ql_np, "qr": qr_np})
    return in_maps


def kernel(theta_log, seed_prob, Ic, c2a):
    assert Ic.shape == (M, NC) and c2a.shape == (NC, K)
    E, Wm, Icf, Z, AT, q = _host_precompute(theta_log, seed_prob, Ic, c2a)
    in_maps = _make_in_maps(Z, AT, q)

    if "nc" not in _cache:
        _cache["nc"] = _build_bass()
    res = run_bass_kernel_spmd(_cache["nc"], in_maps, core_ids=list(range(N_CORES)))

    # device computed sum ln(2^S_EXP * blend) over all (e, j)
    loss_raw = sum(r["loss_pp"].astype(np.float64).sum() for r in res.results)
    loss = -(loss_raw - M * NC * S_EXP * np.log(2.0))
    # row/col sums of S, exact by associativity (f64)
    deg = Wm.sum(axis=0) @ E                     # [NC]
    sizes = Wm @ E.sum(axis=1)                   # [M]
    degree_exp = np.sort(deg)[::-1]
    size_exp = np.sort(sizes)[::-1]
    degree_ans = np.sort(Icf.sum(axis=0))[::-1]
    size_ans = np.sort(Icf.sum(axis=1))[::-1]
    degree_loss = np.mean((degree_exp - degree_ans) ** 2)
    size_loss = np.mean((size_exp - size_ans) ** 2)
    return np.float32(loss + degree_loss + size_loss)


# revision 16
# speedup vs baseline: 1.2547x; 1.0135x over previous
"""Trainium2 Bass kernel for nn_CoreGroupConstruction (segment_reduce).

Reference loss: S = Wm @ exp(P) with Wm row-normalized masked seed weights
([8192, 2048]), P [2048, 2048] edge-independent; pointwise Bernoulli NLL over
all (edge, node) pairs + degree/size moment losses on row/col sums of S.

Algorithmic restructure (vs dense K=2048 matmul): P is bilinear in the K=32
binary attributes: P[i,j] = C + u_i + u_j + sum_k a_ik c_k a_jk, so
E = exp(P) = e^C diag(e^u) (1 + X + O(X^2)) diag(e^u) with X = A diag(c) A^T
of inner rank 32. First-order Taylor in X gives S = Wm @ E as a rank-33
product Z @ AT. The truncation only perturbs terms ~8 orders of magnitude
below the 2e-2 tolerance (masked log S is dominated by the exact host-side
Wm in the blend constant; off-group S ~ 1e-10).

All matmuls run as 128-partition zero-padded fp8 DoubleRow passes (4 rhs
elem/cycle, 256 cycles per 512-wide PSUM pass). Global scale 2^30 lets every
operand sit in fp8 range: Z/AT in e4m3 (alpha=1, beta=e^C*2^30), the blend
constant q' = 2^15*q in e5m2 against a 2^15-scaled identity. The host
de-biases the e5m2 quantization of q exactly (it knows the rounded values),
so the device loss matches the f64 model to ~5e-7.

Per 128-edge tile, PSUM = Z_et @ AT (+ I @ q on PE for blended halves):
 - 6 "pair" tiles: left half blended on DVE (add), multiplied with the
   PE-blended right half, single ACT Ln over the 1024 pair products
   (ln B_L + ln B_R = ln(B_L*B_R)).
 - 2 "direct" tiles: both halves blended on the PE, ACT Lns PSUM directly.
This balances PE ~11us, DVE ~15us, ACT ~14us per core.

Row/col sums of S (degree/size moments) are exact on host by associativity;
host gathers the 8 per-core loss partials and assembles the scalar.
"""

import numpy as np
import ml_dtypes

import concourse.bacc as bacc
import concourse.tile as tile
from concourse import mybir
from concourse.bass_utils import run_bass_kernel_spmd

M, NC, K = 8192, 2048, 32
N_CORES = 8
MLOC = M // N_CORES          # 1024 edges per core
P_DIM = 128
ET = MLOC // P_DIM           # 8 edge tiles per core
NPAIR = 6                    # tiles using the DVE pair-product path
JBLK = 512                   # one f32 PSUM bank
NJ = NC // JBLK              # 4 j-slices
KZ = 64                      # real contraction dim of the low-rank matmul
S_EXP = 30                   # global 2^30 scale
GEXP = 15                    # identity carries 2^15, q' carries 2^(S_EXP-15)
HNC = NC // 2                # pair-product width
NACC = NPAIR + 2 * (ET - NPAIR)   # accumulator columns

_BF16 = ml_dtypes.bfloat16
_DR = None  # set in _build_bass

_cache = {}


def _build_bass():
    nc = bacc.Bacc("TRN2", target_bir_lowering=False, debug=False)
    bf16 = mybir.dt.bfloat16
    e4 = mybir.dt.float8e4
    e5 = mybir.dt.float8e5
    f32 = mybir.dt.float32
    DR = mybir.MatmulPerfMode.DoubleRow

    # DoubleRow layouts: k = r*128 + p; real Z rows live at (r=0, p<64),
    # identity rows at (r=0, all p). Zero padding buys the 4-elem/cycle path.
    zz_d = nc.dram_tensor("zz", [P_DIM, ET, 2, P_DIM], e4, kind="ExternalInput")
    aa_d = nc.dram_tensor("aa", [P_DIM, NJ, 2, JBLK], e4, kind="ExternalInput")
    ii_d = nc.dram_tensor("ii", [P_DIM, 2, P_DIM], e5, kind="ExternalInput")
    ql_d = nc.dram_tensor("ql", [NPAIR, P_DIM, HNC], bf16, kind="ExternalInput")
    qr_d = nc.dram_tensor("qr", [NPAIR, P_DIM, 2, 2, JBLK], e5, kind="ExternalInput")
    qd_d = nc.dram_tensor("qd", [ET - NPAIR, P_DIM, NJ, 2, JBLK], e5,
                          kind="ExternalInput")
    loss_d = nc.dram_tensor("loss_pp", [P_DIM, NACC], f32, kind="ExternalOutput")

    with tile.TileContext(nc) as tc:
        with (
            tc.tile_pool(name="const", bufs=1) as cpool,
            tc.tile_pool(name="qlp", bufs=3) as qlpool,
            tc.tile_pool(name="qrp", bufs=3) as qrpool,
            tc.tile_pool(name="blp", bufs=2) as blpool,
            tc.tile_pool(name="scr", bufs=3) as spool,
            tc.tile_pool(name="psum", bufs=2, space="PSUM") as pspool,
        ):
            loss_pp = cpool.tile([P_DIM, NACC], f32, tag="loss")
            zz_t = cpool.tile([P_DIM, ET, 2, P_DIM], e4, tag="zz")
            aa_t = cpool.tile([P_DIM, NJ, 2, JBLK], e4, tag="aa")
            ii_t = cpool.tile([P_DIM, 2, P_DIM], e5, tag="ii")
            # critical path: zz + left aa half gate the first matmuls
            nc.sync.dma_start(zz_t[:], zz_d[:])
            nc.sync.dma_start(aa_t[:, 0:2], aa_d[:, 0:2])
            nc.sync.dma_start(ii_t[:], ii_d[:])
            nc.sync.dma_start(aa_t[:, 2:NJ], aa_d[:, 2:NJ])

            def zmm(ps, et, jb, stop):
                nc.tensor.matmul(
                    ps[:, (jb % 2) * JBLK:(jb % 2 + 1) * JBLK],
                    zz_t[:, et], aa_t[:, jb],
                    start=True, stop=stop, perf_mode=DR,
                )

            def qmm(ps, qt, jb):
                nc.tensor.matmul(
                    ps[:, (jb % 2) * JBLK:(jb % 2 + 1) * JBLK],
                    ii_t[:], qt,
                    start=False, stop=True, perf_mode=DR,
                )

            for et in range(ET):
                if et < NPAIR:
                    qlt = qlpool.tile([P_DIM, HNC], bf16, tag="ql")
                    nc.gpsimd.dma_start(qlt[:], ql_d[et])
                    qrt = qrpool.tile([P_DIM, 2, 2, JBLK], e5, tag="qr")
                    nc.sync.dma_start(qrt[:], qr_d[et])

                    psl = pspool.tile([P_DIM, HNC], f32, tag="psl")
                    psr = pspool.tile([P_DIM, HNC], f32, tag="psr")
                    for jb in range(2):
                        zmm(psl, et, jb, stop=True)
                    for jb in range(2, NJ):
                        zmm(psr, et, jb, stop=False)
                    for jb in range(2):
                        qmm(psr, qrt[:, jb], jb)
                    # left blend on DVE; pair product ln B_L + ln B_R
                    blt = blpool.tile([P_DIM, HNC], bf16, tag="bl")
                    nc.vector.tensor_add(blt[:], psl[:], qlt[:])
                    scr = spool.tile([P_DIM, HNC], bf16, tag="scr")
                    nc.vector.tensor_mul(scr[:], psr[:], blt[:])
                    nc.scalar.activation(
                        scr[:], scr[:], mybir.ActivationFunctionType.Ln,
                        accum_out=loss_pp[:, et:et + 1],
                    )
                else:
                    ed = et - NPAIR
                    qdt = qrpool.tile([P_DIM, NJ, 2, JBLK], e5, tag="qd")
                    dq = nc.gpsimd if ed % 2 == 0 else nc.sync
                    dq.dma_start(qdt[:], qd_d[ed])

                    psa = pspool.tile([P_DIM, HNC], f32, tag="psl")
                    psb = pspool.tile([P_DIM, HNC], f32, tag="psr")
                    for jb in range(NJ):
                        zmm(psa if jb < 2 else psb, et, jb, stop=False)
                    for jb in range(NJ):
                        qmm(psa if jb < 2 else psb, qdt[:, jb], jb)
                    for half, ps in enumerate((psa, psb)):
                        scr = spool.tile([P_DIM, HNC], bf16, tag="scr")
                        col = NPAIR + 2 * ed + half
                        nc.scalar.activation(
                            scr[:], ps[:], mybir.ActivationFunctionType.Ln,
                            accum_out=loss_pp[:, col:col + 1],
                        )

            nc.sync.dma_start(loss_d[:], loss_pp[:])
    nc.compile()
    return nc


def _host_precompute(theta_log, seed_prob, Ic, c2a):
    theta = -np.logaddexp(0.0, -theta_log.astype(np.float64))  # log_sigmoid [K,3]
    t0, t1, t2 = theta[:, 0], theta[:, 1], theta[:, 2]
    A = c2a.astype(np.float64)
    nA = 1.0 - A
    P = (nA * t0) @ nA.T + (A * t1) @ nA.T + (nA * t1) @ A.T + (A * t2) @ A.T
    np.fill_diagonal(P, 0.0)
    E = np.exp(P)                                # [NC, NC], diag == 1 (exact)

    sp = seed_prob.astype(np.float64)
    seed = np.exp(sp - sp.max())
    seed /= seed.sum()
    Icf = Ic.astype(np.float64)
    rs = Icf @ seed                              # [M]
    Wm = (Icf * seed[None, :]) / rs[:, None]     # [M, NC]

    # rank-33 factorization (see module docstring); alpha=1 on Z
    Cc = t0.sum()
    u = A @ (t1 - t0)
    c = t0 + t2 - 2.0 * t1
    eu = np.exp(u)
    Wt = Wm * eu[None, :]
    Z = np.zeros((M, KZ), np.float64)
    Z[:, 0] = Wt.sum(axis=1)
    Z[:, 1:K + 1] = (Wt @ A) * c[None, :]
    beta = np.exp(Cc) * (2.0 ** S_EXP)
    AT = np.zeros((KZ, NC), np.float64)
    AT[0, :] = eu * beta
    AT[1:K + 1, :] = (A.T * eu[None, :]) * beta

    E1_jj = np.exp(Cc + 2.0 * u) * (1.0 + A @ c)     # Taylor-1 diag of E
    qs = (2.0 ** (S_EXP - GEXP)) * (1.0 - Icf + Wm * (1.0 - E1_jj)[None, :])
    return E, Wm, Icf, Z, AT, qs


def _make_in_maps(Z, AT, qs, Ic):
    e4_np = mybir.dt.np(mybir.dt.float8e4)
    e5_np = mybir.dt.np(mybir.dt.float8e5)
    aa_np = np.zeros((P_DIM, NJ, 2, JBLK), np.float32)
    aa_np[0:KZ, :, 0, :] = AT.reshape(KZ, NJ, JBLK).astype(np.float32)
    aa_np = aa_np.astype(e4_np)
    ii_np = np.zeros((P_DIM, 2, P_DIM), np.float32)
    ii_np[np.arange(P_DIM), 0, np.arange(P_DIM)] = 2.0 ** GEXP
    ii_np = ii_np.astype(e5_np)

    GAM = 2.0 ** GEXP
    in_maps = []
    debias = 0.0
    for cid in range(N_CORES):
        sl = slice(cid * MLOC, (cid + 1) * MLOC)
        Zc = Z[sl]                                   # [1024, KZ]
        zz_np = np.zeros((P_DIM, ET, 2, P_DIM), np.float32)
        zz_np[0:KZ, :, 0, :] = Zc.reshape(ET, P_DIM, KZ).transpose(2, 0, 1)
        zz_np = zz_np.astype(e4_np)

        qc = qs[sl]                                  # q' = 2^15 * q
        icc = Ic[sl]
        npr = NPAIR * P_DIM
        ql_f = (qc[:npr, :HNC] * GAM).astype(_BF16)  # DVE path at 2^30 scale
        ql_np = np.ascontiguousarray(ql_f.reshape(NPAIR, P_DIM, HNC))
        qr_f = qc[:npr, HNC:].astype(e5_np)
        qr_np = np.zeros((NPAIR, P_DIM, 2, 2, JBLK), qr_f.dtype)
        qr_np[:, :, :, 0, :] = qr_f.reshape(NPAIR, P_DIM, 2, JBLK)
        qd_f = qc[npr:, :].astype(e5_np)
        qd_np = np.zeros((ET - NPAIR, P_DIM, NJ, 2, JBLK), qd_f.dtype)
        qd_np[:, :, :, 0, :] = qd_f.reshape(ET - NPAIR, P_DIM, NJ, JBLK)

        # exact de-bias of quantized masked blend constants (host knows the
        # rounded values the device will take ln of)
        mL = icc[:npr, :HNC] == 1
        debias += (np.log(qc[:npr, :HNC][mL] * GAM)
                   - np.log(ql_f.astype(np.float64)[mL])).sum()
        mR = icc[:npr, HNC:] == 1
        debias += (np.log(qc[:npr, HNC:][mR])
                   - np.log(qr_f.astype(np.float64)[mR])).sum()
        mD = icc[npr:, :] == 1
        debias += (np.log(qc[npr:, :][mD])
                   - np.log(qd_f.astype(np.float64)[mD])).sum()

        in_maps.append({"zz": zz_np, "aa": aa_np, "ii": ii_np,
                        "ql": ql_np, "qr": qr_np, "qd": qd_np})
    return in_maps, debias


def kernel(theta_log, seed_prob, Ic, c2a):
    assert Ic.shape == (M, NC) and c2a.shape == (NC, K)
    E, Wm, Icf, Z, AT, qs = _host_precompute(theta_log, seed_prob, Ic, c2a)
    in_maps, debias = _make_in_maps(Z, AT, qs, Ic)

    if "nc" not in _cache:
        _cache["nc"] = _build_bass()
    res = run_bass_kernel_spmd(_cache["nc"], in_maps, core_ids=list(range(N_CORES)))

    # device computed sum ln(2^S_EXP * blend) over all (e, j)
    loss_raw = sum(r["loss_pp"].astype(np.float64).sum() for r in res.results)
    loss = -(loss_raw + debias - M * NC * S_EXP * np.log(2.0))
    # row/col sums of S, exact by associativity (f64)
    deg = Wm.sum(axis=0) @ E                     # [NC]
    sizes = Wm @ E.sum(axis=1)                   # [M]
    degree_exp = np.sort(deg)[::-1]
    size_exp = np.sort(sizes)[::-1]
    degree_ans = np.sort(Icf.sum(axis=0))[::-1]
    size_ans = np.sort(Icf.sum(axis=1))[::-1]
    degree_loss = np.mean((degree_exp - degree_ans) ** 2)
    size_loss = np.mean((size_exp - size_ans) ** 2)
    return np.float32(loss + degree_loss + size_loss)


# revision 17
# speedup vs baseline: 1.3204x; 1.0523x over previous
"""Trainium2 Bass kernel for nn_CoreGroupConstruction (segment_reduce).

Reference loss: S = Wm @ exp(P) with Wm row-normalized masked seed weights
([8192, 2048]), P [2048, 2048] edge-independent; pointwise Bernoulli NLL over
all (edge, node) pairs + degree/size moment losses on row/col sums of S.

Algorithmic restructure (vs dense K=2048 matmul): P is bilinear in the K=32
binary attributes: P[i,j] = C + u_i + u_j + sum_k a_ik c_k a_jk, so
E = exp(P) = e^C diag(e^u) (1 + X + O(X^2)) diag(e^u) with X = A diag(c) A^T
of inner rank 32. First-order Taylor in X gives S = Wm @ E as a rank-33
product Z @ AT. The truncation only perturbs terms ~8 orders of magnitude
below the 2e-2 tolerance (masked log S is dominated by the exact host-side
blend constant; off-group S ~ 1e-10).

Device mapping (per core, 1024 edges, M sharded 8 ways). Global scale 2^30
puts every operand in fp8: Z/AT in e4m3, blend constants q' = 2^15*q in
e5m2 against a 2^15 identity; the host de-biases the e5m2 rounding of q
exactly. Per 128-edge tile and 512-col PSUM bank, TWO chained DoubleRow
matmuls (one accumulation group, no inter-instruction sync):
    PSUM = Z_et @ AT    (e4m3, zero rows at the unused DR slots)
         + I @ q'_et    (e5m2)
The DR pair dimension of each rhs is a stride-0 broadcast (the matching
lhsT rows are zero), so no zero padding is ever shipped: total DMA is
~2.5MB/core. 6 "pair" tiles: DVE copies the left PSUM half to SBUF and
multiplies with the right half, ACT Lns the 1024 products
(ln B_L + ln B_R = ln(B_L*B_R), scale 2^-52 recentres the spline domain);
2 "direct" tiles: ACT Lns both PSUM halves directly (scale 2^-25). This
balances PE ~14us, DVE ~14.5us, ACT ~14us per core.

Row/col sums of S (degree/size moments) are exact on host by associativity;
host gathers the 8 per-core loss partials and assembles the scalar.
"""

import numpy as np
import ml_dtypes

import concourse.bacc as bacc
import concourse.tile as tile
from concourse import mybir
from concourse.bass_utils import run_bass_kernel_spmd

M, NC, K = 8192, 2048, 32
N_CORES = 8
MLOC = M // N_CORES          # 1024 edges per core
P_DIM = 128
ET = MLOC // P_DIM           # 8 edge tiles per core
NPAIR = 6                    # tiles using the DVE pair-product path
JBLK = 512                   # one f32 PSUM bank
NJ = NC // JBLK              # 4 j-slices
KZ = 64                      # real contraction dim of the low-rank matmul
S_EXP = 30                   # global 2^30 scale
GEXP = 15                    # identity carries 2^15, q' carries 2^(S_EXP-15)
HNC = NC // 2                # pair-product width
NACC = NPAIR + 2 * (ET - NPAIR)   # accumulator columns
PAIR_SC = 2.0 ** -52         # Ln input scale for pair products (args ~2^8)
DIR_SC = 2.0 ** -25          # Ln input scale for direct halves (args ~2^5)

_BF16 = ml_dtypes.bfloat16

_cache = {}


def _build_bass():
    nc = bacc.Bacc("TRN2", target_bir_lowering=False, debug=False)
    bf16 = mybir.dt.bfloat16
    e4 = mybir.dt.float8e4
    e5 = mybir.dt.float8e5
    f32 = mybir.dt.float32
    DR = mybir.MatmulPerfMode.DoubleRow

    # DR layouts: k = (p, r); real Z rows at (p<64, r=0), identity at r=0.
    # Each rhs broadcasts its singleton pair dim (stride 0) - the matching
    # lhsT rows are zero, so nothing extra is shipped or stored.
    zz_d = nc.dram_tensor("zz", [P_DIM, ET, 2, P_DIM], e4, kind="ExternalInput")
    aa_d = nc.dram_tensor("aa", [P_DIM, NJ, JBLK], e4, kind="ExternalInput")
    ii_d = nc.dram_tensor("ii", [P_DIM, 2, P_DIM], e5, kind="ExternalInput")
    qq_d = nc.dram_tensor("qq", [ET, P_DIM, NJ, JBLK], e5, kind="ExternalInput")
    loss_d = nc.dram_tensor("loss_pp", [P_DIM, NACC], f32, kind="ExternalOutput")

    with tile.TileContext(nc) as tc:
        with (
            tc.tile_pool(name="const", bufs=1) as cpool,
            tc.tile_pool(name="qp", bufs=4) as qpool,
            tc.tile_pool(name="blp", bufs=2) as blpool,
            tc.tile_pool(name="scr", bufs=3) as spool,
            tc.tile_pool(name="psum", bufs=2, space="PSUM") as pspool,
        ):
            loss_pp = cpool.tile([P_DIM, NACC], f32, tag="loss")
            zz_t = cpool.tile([P_DIM, ET, 2, P_DIM], e4, tag="zz")
            aa_t = cpool.tile([P_DIM, NJ, JBLK], e4, tag="aa")
            ii_t = cpool.tile([P_DIM, 2, P_DIM], e5, tag="ii")
            nc.sync.dma_start(zz_t[:], zz_d[:])
            nc.sync.dma_start(aa_t[:], aa_d[:])
            nc.sync.dma_start(ii_t[:], ii_d[:])

            for et in range(ET):
                qt = qpool.tile([P_DIM, NJ, JBLK], e5, tag="qq")
                dq = nc.gpsimd if et < 4 else nc.sync
                dq.dma_start(qt[:], qq_d[et])

                psl = pspool.tile([P_DIM, HNC], f32, tag="psl")
                psr = pspool.tile([P_DIM, HNC], f32, tag="psr")
                for jb in range(NJ):
                    ps = psl if jb < 2 else psr
                    sl = slice((jb % 2) * JBLK, (jb % 2 + 1) * JBLK)
                    nc.tensor.matmul(
                        ps[:, sl], zz_t[:, et],
                        aa_t[:, jb:jb + 1, :].to_broadcast([P_DIM, 2, JBLK]),
                        start=True, stop=False, perf_mode=DR,
                    )
                    nc.tensor.matmul(
                        ps[:, sl], ii_t[:],
                        qt[:, jb:jb + 1, :].to_broadcast([P_DIM, 2, JBLK]),
                        start=False, stop=True, perf_mode=DR,
                    )
                if et < NPAIR:
                    blt = blpool.tile([P_DIM, HNC], bf16, tag="bl")
                    nc.vector.tensor_copy(blt[:], psl[:])
                    scr = spool.tile([P_DIM, HNC], bf16, tag="scr")
                    nc.vector.tensor_mul(scr[:], psr[:], blt[:])
                    nc.scalar.activation(
                        scr[:], scr[:], mybir.ActivationFunctionType.Ln,
                        scale=PAIR_SC,
                        accum_out=loss_pp[:, et:et + 1],
                    )
                else:
                    ed = et - NPAIR
                    for half, ps in enumerate((psl, psr)):
                        scr = spool.tile([P_DIM, HNC], bf16, tag="scr")
                        col = NPAIR + 2 * ed + half
                        nc.scalar.activation(
                            scr[:], ps[:], mybir.ActivationFunctionType.Ln,
                            scale=DIR_SC,
                            accum_out=loss_pp[:, col:col + 1],
                        )

            nc.sync.dma_start(loss_d[:], loss_pp[:])
    nc.compile()
    return nc


def _host_precompute(theta_log, seed_prob, Ic, c2a):
    theta = -np.logaddexp(0.0, -theta_log.astype(np.float64))  # log_sigmoid [K,3]
    t0, t1, t2 = theta[:, 0], theta[:, 1], theta[:, 2]
    A = c2a.astype(np.float64)
    nA = 1.0 - A
    P = (nA * t0) @ nA.T + (A * t1) @ nA.T + (nA * t1) @ A.T + (A * t2) @ A.T
    np.fill_diagonal(P, 0.0)
    E = np.exp(P)                                # [NC, NC], diag == 1 (exact)

    sp = seed_prob.astype(np.float64)
    seed = np.exp(sp - sp.max())
    seed /= seed.sum()
    Icf = Ic.astype(np.float64)
    rs = Icf @ seed                              # [M]
    Wm = (Icf * seed[None, :]) / rs[:, None]     # [M, NC]

    # rank-33 factorization (see module docstring); alpha=1 on Z
    Cc = t0.sum()
    u = A @ (t1 - t0)
    c = t0 + t2 - 2.0 * t1
    eu = np.exp(u)
    Wt = Wm * eu[None, :]
    Z = np.zeros((M, KZ), np.float64)
    Z[:, 0] = Wt.sum(axis=1)
    Z[:, 1:K + 1] = (Wt @ A) * c[None, :]
    beta = np.exp(Cc) * (2.0 ** S_EXP)
    AT = np.zeros((KZ, NC), np.float64)
    AT[0, :] = eu * beta
    AT[1:K + 1, :] = (A.T * eu[None, :]) * beta

    E1_jj = np.exp(Cc + 2.0 * u) * (1.0 + A @ c)     # Taylor-1 diag of E
    qs = (2.0 ** (S_EXP - GEXP)) * (1.0 - Icf + Wm * (1.0 - E1_jj)[None, :])
    return E, Wm, Icf, Z, AT, qs


def _make_in_maps(Z, AT, qs, Ic):
    e4_np = mybir.dt.np(mybir.dt.float8e4)
    e5_np = mybir.dt.np(mybir.dt.float8e5)
    aa_np = np.zeros((P_DIM, NJ, JBLK), np.float32)
    aa_np[0:KZ] = AT.reshape(KZ, NJ, JBLK).astype(np.float32)
    aa_np = aa_np.astype(e4_np)
    ii_np = np.zeros((P_DIM, 2, P_DIM), np.float32)
    ii_np[np.arange(P_DIM), 0, np.arange(P_DIM)] = 2.0 ** GEXP
    ii_np = ii_np.astype(e5_np)

    in_maps = []
    debias = 0.0
    for cid in range(N_CORES):
        sl = slice(cid * MLOC, (cid + 1) * MLOC)
        zz_np = np.zeros((P_DIM, ET, 2, P_DIM), np.float32)
        zz_np[0:KZ, :, 0, :] = Z[sl].reshape(ET, P_DIM, KZ).transpose(2, 0, 1)
        zz_np = zz_np.astype(e4_np)

        qc = qs[sl]                                  # q' = 2^15 * q
        qq_f = qc.astype(e5_np)
        qq_np = np.ascontiguousarray(
            qq_f.reshape(ET, P_DIM, NJ, JBLK))

        mask = Ic[sl] == 1
        debias += (np.log(qc[mask])
                   - np.log(qq_f.astype(np.float64)[mask])).sum()

        in_maps.append({"zz": zz_np, "aa": aa_np, "ii": ii_np, "qq": qq_np})
    return in_maps, debias


def kernel(theta_log, seed_prob, Ic, c2a):
    assert Ic.shape == (M, NC) and c2a.shape == (NC, K)
    E, Wm, Icf, Z, AT, qs = _host_precompute(theta_log, seed_prob, Ic, c2a)
    in_maps, debias = _make_in_maps(Z, AT, qs, Ic)

    if "nc" not in _cache:
        _cache["nc"] = _build_bass()
    res = run_bass_kernel_spmd(_cache["nc"], in_maps, core_ids=list(range(N_CORES)))

    # device: pair cols accumulated ln(2^-52 * B_L*B_R), direct cols
    # ln(2^-25 * B), B = 2^30 * blend
    loss_raw = sum(r["loss_pp"].astype(np.float64).sum() for r in res.results)
    n_pair = N_CORES * NPAIR * P_DIM * HNC           # pairs (2 elements each)
    n_dir = N_CORES * (ET - NPAIR) * P_DIM * NC      # single elements
    lconst = (n_pair * (2 * S_EXP - 52) + n_dir * (S_EXP - 25)) * np.log(2.0)
    loss = -(loss_raw + debias - lconst)
    # row/col sums of S, exact by associativity (f64)
    deg = Wm.sum(axis=0) @ E                     # [NC]
    sizes = Wm @ E.sum(axis=1)                   # [M]
    degree_exp = np.sort(deg)[::-1]
    size_exp = np.sort(sizes)[::-1]
    degree_ans = np.sort(Icf.sum(axis=0))[::-1]
    size_ans = np.sort(Icf.sum(axis=1))[::-1]
    degree_loss = np.mean((degree_exp - degree_ans) ** 2)
    size_loss = np.mean((size_exp - size_ans) ** 2)
    return np.float32(loss + degree_loss + size_loss)


# revision 18
# speedup vs baseline: 1.4296x; 1.0827x over previous
"""Trainium2 Bass kernel for nn_CoreGroupConstruction (segment_reduce).

Reference loss: S = Wm @ exp(P) with Wm row-normalized masked seed weights
([8192, 2048]), P [2048, 2048] edge-independent; pointwise Bernoulli NLL over
all (edge, node) pairs + degree/size moment losses on row/col sums of S.

Algorithmic restructure (vs dense K=2048 matmul): P is bilinear in the K=32
binary attributes: P[i,j] = C + u_i + u_j + sum_k a_ik c_k a_jk, so
E = exp(P) = e^C diag(e^u) (1 + X + O(X^2)) diag(e^u) with X = A diag(c) A^T
of inner rank 32. First-order Taylor in X gives S = Wm @ E as a rank-33
product Z @ AT. The truncation only perturbs terms ~8 orders of magnitude
below the 2e-2 tolerance (masked log S is dominated by the exact host-side
blend constant; off-group S ~ 1e-10).

Device mapping (per core, 1024 edges, M sharded 8 ways). Global scale 2^30
puts every operand in fp8: Z/AT in e4m3, blend constants q' = 2^15*q in
e5m2 against a 2^15 identity; the host de-biases the e5m2 rounding of q
exactly. Per 128-edge tile and 512-col PSUM bank, TWO chained DoubleRow
matmuls (one accumulation group, no inter-instruction sync):
    PSUM = Z_et @ AT    (e4m3, zero rows at the unused DR slots)
         + I @ q'_et    (e5m2)
The DR pair dimension of each rhs is a stride-0 broadcast (the matching
lhsT rows are zero), so no zero padding is ever shipped: total DMA is
~2.5MB/core. 6 "pair" tiles: DVE copies the left PSUM half to SBUF and
multiplies with the right half, ACT Lns the 1024 products
(ln B_L + ln B_R = ln(B_L*B_R), scale 2^-52 recentres the spline domain);
2 "direct" tiles: ACT Lns both PSUM halves directly (scale 2^-25). This
balances PE ~14us, DVE ~14.5us, ACT ~14us per core.

Row/col sums of S (degree/size moments) are exact on host by associativity;
host gathers the 8 per-core loss partials and assembles the scalar.
"""

import numpy as np
import ml_dtypes

import concourse.bacc as bacc
import concourse.tile as tile
from concourse import mybir
from concourse.bass_utils import run_bass_kernel_spmd

M, NC, K = 8192, 2048, 32
N_CORES = 8
MLOC = M // N_CORES          # 1024 edges per core
P_DIM = 128
ET = MLOC // P_DIM           # 8 edge tiles per core
NPAIR = 6                    # tiles using the DVE pair-product path
JBLK = 512                   # one f32 PSUM bank
NJ = NC // JBLK              # 4 j-slices
KZ = 64                      # real contraction dim of the low-rank matmul
S_EXP = 30                   # global 2^30 scale
GEXP = 15                    # identity carries 2^15, q' carries 2^(S_EXP-15)
HNC = NC // 2                # pair-product width
NACC = NPAIR + 2 * (ET - NPAIR)   # accumulator columns
PAIR_SC = 2.0 ** -52         # Ln input scale for pair products (args ~2^8)
DIR_SC = 2.0 ** -25          # Ln input scale for direct halves (args ~2^5)

_BF16 = ml_dtypes.bfloat16

_cache = {}


def _build_bass():
    nc = bacc.Bacc("TRN2", target_bir_lowering=False, debug=False)
    bf16 = mybir.dt.bfloat16
    e4 = mybir.dt.float8e4
    e5 = mybir.dt.float8e5
    f32 = mybir.dt.float32
    DR = mybir.MatmulPerfMode.DoubleRow

    # DR layouts: k = (p, r); real Z rows at (p<64, r=0), identity at r=0.
    # Each rhs broadcasts its singleton pair dim (stride 0) - the matching
    # lhsT rows are zero, so nothing extra is shipped or stored.
    zz_d = nc.dram_tensor("zz", [P_DIM, ET, 2, P_DIM], e4, kind="ExternalInput")
    aa_d = nc.dram_tensor("aa", [P_DIM, NJ, JBLK], e4, kind="ExternalInput")
    ii_d = nc.dram_tensor("ii", [P_DIM, 2, P_DIM], e5, kind="ExternalInput")
    qq_d = nc.dram_tensor("qq", [ET, P_DIM, NJ, JBLK], e5, kind="ExternalInput")
    loss_d = nc.dram_tensor("loss_pp", [P_DIM, NACC], f32, kind="ExternalOutput")

    with tile.TileContext(nc) as tc:
        with (
            tc.tile_pool(name="const", bufs=1) as cpool,
            tc.tile_pool(name="qp", bufs=4) as qpool,
            tc.tile_pool(name="blp", bufs=2) as blpool,
            tc.tile_pool(name="scr", bufs=3) as spool,
            tc.tile_pool(name="psum", bufs=2, space="PSUM") as pspool,
        ):
            loss_pp = cpool.tile([P_DIM, NACC], f32, tag="loss")
            zz_t = cpool.tile([P_DIM, ET, 2, P_DIM], e4, tag="zz")
            aa_t = cpool.tile([P_DIM, NJ, JBLK], e4, tag="aa")
            ii_t = cpool.tile([P_DIM, 2, P_DIM], e5, tag="ii")
            nc.sync.dma_start(zz_t[:], zz_d[:])
            nc.gpsimd.dma_start(aa_t[:], aa_d[:])
            nc.sync.dma_start(ii_t[:], ii_d[:])

            # schedule: direct tiles (2 ACTs, no DVE) sit at positions 2 and
            # 7 so their ACT work overlaps the stream / shortens the tail;
            # position 4 uses the ACT-copy variant to shed DVE time.
            DIRECT = (2, 7)
            ACOPY = (4,)
            acc_col = iter(range(NACC))

            for et in range(ET):
                qt = qpool.tile([P_DIM, NJ, JBLK], e5, tag="qq")
                dq = nc.gpsimd if et % 2 == 0 else nc.sync
                dq.dma_start(qt[:], qq_d[et])

                psl = pspool.tile([P_DIM, HNC], f32, tag="psl")
                psr = pspool.tile([P_DIM, HNC], f32, tag="psr")

                def bank(ps, jb):
                    sl = slice((jb % 2) * JBLK, (jb % 2 + 1) * JBLK)
                    nc.tensor.matmul(
                        ps[:, sl], zz_t[:, et],
                        aa_t[:, jb:jb + 1, :].to_broadcast([P_DIM, 2, JBLK]),
                        start=True, stop=False, perf_mode=DR,
                    )
                    nc.tensor.matmul(
                        ps[:, sl], ii_t[:],
                        qt[:, jb:jb + 1, :].to_broadcast([P_DIM, 2, JBLK]),
                        start=False, stop=True, perf_mode=DR,
                    )

                if et in DIRECT:
                    # emit each half's Ln right after its banks so the ACT
                    # overlaps the remaining matmuls
                    for half, ps in enumerate((psl, psr)):
                        for jb in (2 * half, 2 * half + 1):
                            bank(ps, jb)
                        scr = spool.tile([P_DIM, HNC], bf16, tag="scr")
                        col = next(acc_col)
                        nc.scalar.activation(
                            scr[:], ps[:], mybir.ActivationFunctionType.Ln,
                            scale=DIR_SC,
                            accum_out=loss_pp[:, col:col + 1],
                        )
                else:
                    for jb in range(NJ):
                        bank(psl if jb < 2 else psr, jb)
                    blt = blpool.tile([P_DIM, HNC], bf16, tag="bl")
                    if et in ACOPY:
                        nc.scalar.activation(
                            blt[:], psl[:],
                            mybir.ActivationFunctionType.Copy,
                        )
                    else:
                        nc.vector.tensor_copy(blt[:], psl[:])
                    scr = spool.tile([P_DIM, HNC], bf16, tag="scr")
                    nc.vector.tensor_mul(scr[:], psr[:], blt[:])
                    col = next(acc_col)
                    nc.scalar.activation(
                        scr[:], scr[:], mybir.ActivationFunctionType.Ln,
                        scale=PAIR_SC,
                        accum_out=loss_pp[:, col:col + 1],
                    )

            nc.sync.dma_start(loss_d[:], loss_pp[:])
    nc.compile()
    return nc


def _host_precompute(theta_log, seed_prob, Ic, c2a):
    theta = -np.logaddexp(0.0, -theta_log.astype(np.float64))  # log_sigmoid [K,3]
    t0, t1, t2 = theta[:, 0], theta[:, 1], theta[:, 2]
    A = c2a.astype(np.float64)
    nA = 1.0 - A
    P = (nA * t0) @ nA.T + (A * t1) @ nA.T + (nA * t1) @ A.T + (A * t2) @ A.T
    np.fill_diagonal(P, 0.0)
    E = np.exp(P)                                # [NC, NC], diag == 1 (exact)

    sp = seed_prob.astype(np.float64)
    seed = np.exp(sp - sp.max())
    seed /= seed.sum()
    Icf = Ic.astype(np.float64)
    rs = Icf @ seed                              # [M]
    Wm = (Icf * seed[None, :]) / rs[:, None]     # [M, NC]

    # rank-33 factorization (see module docstring); alpha=1 on Z
    Cc = t0.sum()
    u = A @ (t1 - t0)
    c = t0 + t2 - 2.0 * t1
    eu = np.exp(u)
    Wt = Wm * eu[None, :]
    Z = np.zeros((M, KZ), np.float64)
    Z[:, 0] = Wt.sum(axis=1)
    Z[:, 1:K + 1] = (Wt @ A) * c[None, :]
    beta = np.exp(Cc) * (2.0 ** S_EXP)
    AT = np.zeros((KZ, NC), np.float64)
    AT[0, :] = eu * beta
    AT[1:K + 1, :] = (A.T * eu[None, :]) * beta

    E1_jj = np.exp(Cc + 2.0 * u) * (1.0 + A @ c)     # Taylor-1 diag of E
    qs = (2.0 ** (S_EXP - GEXP)) * (1.0 - Icf + Wm * (1.0 - E1_jj)[None, :])
    return E, Wm, Icf, Z, AT, qs


def _make_in_maps(Z, AT, qs, Ic):
    e4_np = mybir.dt.np(mybir.dt.float8e4)
    e5_np = mybir.dt.np(mybir.dt.float8e5)
    aa_np = np.zeros((P_DIM, NJ, JBLK), np.float32)
    aa_np[0:KZ] = AT.reshape(KZ, NJ, JBLK).astype(np.float32)
    aa_np = aa_np.astype(e4_np)
    ii_np = np.zeros((P_DIM, 2, P_DIM), np.float32)
    ii_np[np.arange(P_DIM), 0, np.arange(P_DIM)] = 2.0 ** GEXP
    ii_np = ii_np.astype(e5_np)

    in_maps = []
    debias = 0.0
    for cid in range(N_CORES):
        sl = slice(cid * MLOC, (cid + 1) * MLOC)
        zz_np = np.zeros((P_DIM, ET, 2, P_DIM), np.float32)
        zz_np[0:KZ, :, 0, :] = Z[sl].reshape(ET, P_DIM, KZ).transpose(2, 0, 1)
        zz_np = zz_np.astype(e4_np)

        qc = qs[sl]                                  # q' = 2^15 * q
        qq_f = qc.astype(e5_np)
        qq_np = np.ascontiguousarray(
            qq_f.reshape(ET, P_DIM, NJ, JBLK))

        mask = Ic[sl] == 1
        debias += (np.log(qc[mask])
                   - np.log(qq_f.astype(np.float64)[mask])).sum()

        in_maps.append({"zz": zz_np, "aa": aa_np, "ii": ii_np, "qq": qq_np})
    return in_maps, debias


def kernel(theta_log, seed_prob, Ic, c2a):
    assert Ic.shape == (M, NC) and c2a.shape == (NC, K)
    E, Wm, Icf, Z, AT, qs = _host_precompute(theta_log, seed_prob, Ic, c2a)
    in_maps, debias = _make_in_maps(Z, AT, qs, Ic)

    if "nc" not in _cache:
        _cache["nc"] = _build_bass()
    res = run_bass_kernel_spmd(_cache["nc"], in_maps, core_ids=list(range(N_CORES)))

    # device: pair cols accumulated ln(2^-52 * B_L*B_R), direct cols
    # ln(2^-25 * B), B = 2^30 * blend
    loss_raw = sum(r["loss_pp"].astype(np.float64).sum() for r in res.results)
    n_pair = N_CORES * NPAIR * P_DIM * HNC           # pairs (2 elements each)
    n_dir = N_CORES * (ET - NPAIR) * P_DIM * NC      # single elements
    lconst = (n_pair * (2 * S_EXP - 52) + n_dir * (S_EXP - 25)) * np.log(2.0)
    loss = -(loss_raw + debias - lconst)
    # row/col sums of S, exact by associativity (f64)
    deg = Wm.sum(axis=0) @ E                     # [NC]
    sizes = Wm @ E.sum(axis=1)                   # [M]
    degree_exp = np.sort(deg)[::-1]
    size_exp = np.sort(sizes)[::-1]
    degree_ans = np.sort(Icf.sum(axis=0))[::-1]
    size_ans = np.sort(Icf.sum(axis=1))[::-1]
    degree_loss = np.mean((degree_exp - degree_ans) ** 2)
    size_loss = np.mean((size_exp - size_ans) ** 2)
    return np.float32(loss + degree_loss + size_loss)


# revision 25
# speedup vs baseline: 1.4710x; 1.0290x over previous
"""Trainium2 Bass kernel for nn_CoreGroupConstruction (segment_reduce).

Reference loss: S = Wm @ exp(P) with Wm row-normalized masked seed weights
([8192, 2048]), P [2048, 2048] edge-independent; pointwise Bernoulli NLL over
all (edge, node) pairs + degree/size moment losses on row/col sums of S.

Algorithmic restructure (vs dense K=2048 matmul): P is bilinear in the K=32
binary attributes: P[i,j] = C + u_i + u_j + sum_k a_ik c_k a_jk, so
E = exp(P) = e^C diag(e^u) (1 + X + O(X^2)) diag(e^u) with X = A diag(c) A^T
of inner rank 32. First-order Taylor in X gives S = Wm @ E as a rank-33
product Z @ AT. The truncation only perturbs terms ~8 orders of magnitude
below the 2e-2 tolerance (masked log S is dominated by the exact host-side
blend constant; off-group S ~ 1e-10).

Device mapping (per core, 1024 edges, M sharded 8 ways). Global scale 2^30
puts every operand in fp8: Z/AT in e4m3, blend constants q' = 2^15*q in
e5m2 against a 2^15 identity; the host de-biases the e5m2 rounding of q
exactly. Per 128-edge tile and 512-col PSUM bank, TWO chained DoubleRow
matmuls (one accumulation group, no inter-instruction sync):
    PSUM = Z_et @ AT    (e4m3, zero rows at the unused DR slots)
         + I @ q'_et    (e5m2)
The DR pair dimension of each rhs is a stride-0 broadcast (the matching
lhsT rows are zero), so no zero padding is ever shipped: total DMA is
~2.5MB/core. 6 "pair" tiles: DVE copies the left PSUM half to SBUF and
multiplies with the right half, ACT Lns the 1024 products
(ln B_L + ln B_R = ln(B_L*B_R), scale 2^-52 recentres the spline domain);
2 "direct" tiles: ACT Lns both PSUM halves directly (scale 2^-25). This
balances PE ~14us, DVE ~14.5us, ACT ~14us per core.

Row/col sums of S (degree/size moments) are exact on host by associativity;
host gathers the 8 per-core loss partials and assembles the scalar.
"""

import numpy as np
import ml_dtypes

import concourse.bacc as bacc
import concourse.tile as tile
from concourse import mybir
from concourse.bass_utils import run_bass_kernel_spmd

M, NC, K = 8192, 2048, 32
N_CORES = 8
MLOC = M // N_CORES          # 1024 edges per core
P_DIM = 128
ET = MLOC // P_DIM           # 8 edge tiles per core
NPAIR = 6                    # tiles using the DVE pair-product path
JBLK = 512                   # one f32 PSUM bank
NJ = NC // JBLK              # 4 j-slices
KZ = 64                      # real contraction dim of the low-rank matmul
S_EXP = 30                   # global 2^30 scale
GEXP = 15                    # identity carries 2^15, q' carries 2^(S_EXP-15)
HNC = NC // 2                # pair-product width
NACC = NPAIR + 2 * (ET - NPAIR)   # accumulator columns
PAIR_SC = 2.0 ** -52         # Ln input scale for pair products (args ~2^8)
DIR_SC = 2.0 ** -25          # Ln input scale for direct halves (args ~2^5)

_BF16 = ml_dtypes.bfloat16

_cache = {}


def _build_bass():
    nc = bacc.Bacc("TRN2", target_bir_lowering=False, debug=False)
    bf16 = mybir.dt.bfloat16
    e4 = mybir.dt.float8e4
    e5 = mybir.dt.float8e5
    f32 = mybir.dt.float32
    DR = mybir.MatmulPerfMode.DoubleRow

    # Combined DR layout, k-slot = (p, r): Z columns at (p<64, r=0),
    # identity rows for edges 0-63 at (p>=64, r=0) and edges 64-127 at
    # (p<64, r=1). One DoubleRow matmul per PSUM bank computes
    # Z_et @ AT + 2^15 * I @ q' in a single instruction: the rhs tile
    # interleaves AT rows with the q' tile rows in the same slot layout.
    zz_d = nc.dram_tensor("zz", [P_DIM, ET, 2, P_DIM], e5, kind="ExternalInput")
    qa_d = nc.dram_tensor("qa", [ET, P_DIM, NJ, 2, JBLK], e5, kind="ExternalInput")
    loss_d = nc.dram_tensor("loss_pp", [P_DIM, NACC], f32, kind="ExternalOutput")

    with tile.TileContext(nc) as tc:
        with (
            tc.tile_pool(name="const", bufs=1) as cpool,
            tc.tile_pool(name="qp", bufs=4) as qpool,
            tc.tile_pool(name="blp", bufs=2) as blpool,
            tc.tile_pool(name="scr", bufs=3) as spool,
            tc.tile_pool(name="psum", bufs=2, space="PSUM") as pspool,
        ):
            loss_pp = cpool.tile([P_DIM, NACC], f32, tag="loss")
            zz_t = cpool.tile([P_DIM, ET, 2, P_DIM], e5, tag="zz")
            # first tile's weights land first (small DMA gates first matmul)
            nc.sync.dma_start(zz_t[:, 0:1], zz_d[:, 0:1])
            nc.sync.dma_start(zz_t[:, 1:ET], zz_d[:, 1:ET])

            # schedule: direct tiles (2 ACTs, no DVE) sit at positions 2 and
            # 7 so their ACT work overlaps the stream / shortens the tail;
            # position 4 uses the ACT-copy variant to shed DVE time.
            DIRECT = (2, 7)
            ACOPY = (4,)
            acc_col = iter(range(NACC))

            for et in range(ET):
                qt = qpool.tile([P_DIM, NJ, 2, JBLK], e5, tag="qq")
                dq = nc.gpsimd if et % 2 == 0 else nc.sync
                dq.dma_start(qt[:], qa_d[et])

                psl = pspool.tile([P_DIM, HNC], f32, tag="psl")
                psr = pspool.tile([P_DIM, HNC], f32, tag="psr")

                def bank(ps, jb):
                    sl = slice((jb % 2) * JBLK, (jb % 2 + 1) * JBLK)
                    nc.tensor.matmul(
                        ps[:, sl], zz_t[:, et], qt[:, jb],
                        start=True, stop=True, perf_mode=DR,
                    )

                if et in DIRECT:
                    # emit each half's Ln right after its banks so the ACT
                    # overlaps the remaining matmuls
                    for half, ps in enumerate((psl, psr)):
                        for jb in (2 * half, 2 * half + 1):
                            bank(ps, jb)
                        scr = spool.tile([P_DIM, HNC], bf16, tag="scr")
                        col = next(acc_col)
                        nc.scalar.activation(
                            scr[:], ps[:], mybir.ActivationFunctionType.Ln,
                            scale=DIR_SC,
                            accum_out=loss_pp[:, col:col + 1],
                        )
                else:
                    for jb in range(NJ):
                        bank(psl if jb < 2 else psr, jb)
                    blt = blpool.tile([P_DIM, HNC], bf16, tag="bl")
                    if et in ACOPY:
                        nc.scalar.activation(
                            blt[:], psl[:],
                            mybir.ActivationFunctionType.Copy,
                        )
                    else:
                        nc.vector.tensor_copy(blt[:], psl[:])
                    scr = spool.tile([P_DIM, HNC], bf16, tag="scr")
                    nc.vector.tensor_mul(scr[:], psr[:], blt[:])
                    col = next(acc_col)
                    nc.scalar.activation(
                        scr[:], scr[:], mybir.ActivationFunctionType.Ln,
                        scale=PAIR_SC,
                        accum_out=loss_pp[:, col:col + 1],
                    )

            # ship the early columns while the last tile's Ln still runs
            nc.sync.dma_start(loss_d[:, 0:NACC - 1], loss_pp[:, 0:NACC - 1])
            nc.sync.dma_start(loss_d[:, NACC - 1:], loss_pp[:, NACC - 1:])
    nc.compile()
    return nc


def _host_precompute(theta_log, seed_prob, Ic, c2a):
    theta = -np.logaddexp(0.0, -theta_log.astype(np.float64))  # log_sigmoid [K,3]
    t0, t1, t2 = theta[:, 0], theta[:, 1], theta[:, 2]
    A = c2a.astype(np.float64)
    nA = 1.0 - A
    P = (nA * t0) @ nA.T + (A * t1) @ nA.T + (nA * t1) @ A.T + (A * t2) @ A.T
    np.fill_diagonal(P, 0.0)
    E = np.exp(P)                                # [NC, NC], diag == 1 (exact)

    sp = seed_prob.astype(np.float64)
    seed = np.exp(sp - sp.max())
    seed /= seed.sum()
    Icf = Ic.astype(np.float64)
    rs = Icf @ seed                              # [M]
    Wm = (Icf * seed[None, :]) / rs[:, None]     # [M, NC]

    # rank-33 factorization (see module docstring); alpha=1 on Z
    Cc = t0.sum()
    u = A @ (t1 - t0)
    c = t0 + t2 - 2.0 * t1
    eu = np.exp(u)
    Wt = Wm * eu[None, :]
    Z = np.zeros((M, KZ), np.float64)
    Z[:, 0] = Wt.sum(axis=1)
    Z[:, 1:K + 1] = (Wt @ A) * c[None, :]
    beta = np.exp(Cc) * (2.0 ** S_EXP)
    AT = np.zeros((KZ, NC), np.float64)
    AT[0, :] = eu * beta
    AT[1:K + 1, :] = (A.T * eu[None, :]) * beta

    E1_jj = np.exp(Cc + 2.0 * u) * (1.0 + A @ c)     # Taylor-1 diag of E
    qs = (2.0 ** (S_EXP - GEXP)) * (1.0 - Icf + Wm * (1.0 - E1_jj)[None, :])
    return E, Wm, Icf, Z, AT, qs


def _make_in_maps(Z, AT, qs, Ic):
    e5_np = mybir.dt.np(mybir.dt.float8e5)
    at_f = AT.reshape(KZ, NJ, JBLK).astype(np.float32)   # shared rhs rows

    in_maps = []
    debias = 0.0
    for cid in range(N_CORES):
        sl = slice(cid * MLOC, (cid + 1) * MLOC)
        # lhsT: Z columns at (p<64, r=0); identity 2^15 for edges 0-63 at
        # (p>=64, r=0) and edges 64-127 at (p<64, r=1)
        zz_np = np.zeros((P_DIM, ET, 2, P_DIM), np.float32)
        zz_np[0:KZ, :, 0, :] = Z[sl].reshape(ET, P_DIM, KZ).transpose(2, 0, 1)
        for e in range(KZ):
            zz_np[KZ + e, :, 0, e] = 2.0 ** GEXP
            zz_np[e, :, 1, KZ + e] = 2.0 ** GEXP
        zz_np = zz_np.astype(e5_np)

        qc = qs[sl]                                  # q' = 2^15 * q
        qq_f = qc.astype(e5_np)
        # rhs: AT rows + this tile's q' rows in the matching slot layout
        qh = qq_f.reshape(ET, 2, KZ, NJ, JBLK)       # [et, half, e, jb, j]
        qa_np = np.zeros((ET, P_DIM, NJ, 2, JBLK), e5_np)
        qa_np[:, 0:KZ, :, 0, :] = at_f.astype(e5_np)[None]
        qa_np[:, KZ:, :, 0, :] = qh[:, 0]
        qa_np[:, 0:KZ, :, 1, :] = qh[:, 1]

        mask = Ic[sl] == 1
        debias += (np.log(qc[mask])
                   - np.log(qq_f.astype(np.float64)[mask])).sum()

        in_maps.append({"zz": zz_np, "qa": qa_np})
    return in_maps, debias


def kernel(theta_log, seed_prob, Ic, c2a):
    assert Ic.shape == (M, NC) and c2a.shape == (NC, K)
    E, Wm, Icf, Z, AT, qs = _host_precompute(theta_log, seed_prob, Ic, c2a)
    in_maps, debias = _make_in_maps(Z, AT, qs, Ic)

    if "nc" not in _cache:
        _cache["nc"] = _build_bass()
    res = run_bass_kernel_spmd(_cache["nc"], in_maps, core_ids=list(range(N_CORES)))

    # device: pair cols accumulated ln(2^-52 * B_L*B_R), direct cols
    # ln(2^-25 * B), B = 2^30 * blend
    loss_raw = sum(r["loss_pp"].astype(np.float64).sum() for r in res.results)
    n_pair = N_CORES * NPAIR * P_DIM * HNC           # pairs (2 elements each)
    n_dir = N_CORES * (ET - NPAIR) * P_DIM * NC      # single elements
    lconst = (n_pair * (2 * S_EXP - 52) + n_dir * (S_EXP - 25)) * np.log(2.0)
    loss = -(loss_raw + debias - lconst)
    # row/col sums of S, exact by associativity (f64)
    deg = Wm.sum(axis=0) @ E                     # [NC]
    sizes = Wm @ E.sum(axis=1)                   # [M]
    degree_exp = np.sort(deg)[::-1]
    size_exp = np.sort(sizes)[::-1]
    degree_ans = np.sort(Icf.sum(axis=0))[::-1]
    size_ans = np.sort(Icf.sum(axis=1))[::-1]
    degree_loss = np.mean((degree_exp - degree_ans) ** 2)
    size_loss = np.mean((size_exp - size_ans) ** 2)
    return np.float32(loss + degree_loss + size_loss)
